# revision 18
# baseline (speedup 1.0000x reference)
"""Trainium2 Bass kernel for CachedRoPEAttention.

Sharding: 8 cores = batch(2) x head-groups(4). Each core computes 4 heads of
one batch element end-to-end (q/k/v proj in [e,t] layout, RoPE, causal
flash-style attention with ones-row softmax denominators, out_proj partial),
host sums the 4 tensor-parallel partials per batch.

All matmuls run in bf16 (inputs host-cast; PSUM accumulation stays f32).
Key perf structure vs the f32r baseline:
  - bf16 halves HBM traffic and keeps the PE at 1 cycle/row for every n.
  - v is produced as vT via weight-stationary matmuls (n=512, LDWEIGHTS
    hidden), then flipped to [t, e] layout by XBAR DMA transposes -- no PE
    or DVE cost.
  - The RoPE partition shuffle rides the final DVE add (cross-partition
    in1 offsets) instead of serial SBUF-SBUF DMAs on the sync queue.
  - Softmax denominators: DVE reciprocal_approx_fast on the [1,512] ones
    rows + DMA partition-broadcast, replacing the 3.3us full-precision
    reciprocals and the PE ones-outer-product broadcast.
  - x/w loads are issued in consumption order (wq, x-slab0, wk, ...) so the
    first projection matmul starts as early as possible.
"""
import sys
sys.path.insert(0, "/opt/trn_rl_repo")

import numpy as np

import concourse.bass as bass
import concourse.bacc as bacc
import concourse.mybir as mybir
import concourse.tile as tile
from concourse.bass_utils import run_bass_kernel_spmd

F32 = mybir.dt.float32
BF16 = mybir.dt.bfloat16

D, H, DH, T, B = 1024, 16, 64, 2048, 2
HG, HPC, EC = 4, 4, 256      # head groups, heads/core, e-width/core
KT = D // 128                # 8 contraction tiles over d_model
PT = EC // 128               # 2 e-partition-tiles (head pairs) per core
NB = T // 512                # 4 t-blocks
NTT = T // 128               # 16 t-tiles

_NC_CACHE = {}


def _build_nc():
    nc = bacc.Bacc(None, target_bir_lowering=False)

    xT_d = nc.dram_tensor("xT", [D, T], BF16, kind="ExternalInput")
    wqT_d = nc.dram_tensor("wqT", [D, EC], BF16, kind="ExternalInput")
    wkT_d = nc.dram_tensor("wkT", [D, EC], BF16, kind="ExternalInput")
    wvT_d = nc.dram_tensor("wvT", [D, EC], BF16, kind="ExternalInput")
    woT_d = nc.dram_tensor("woT", [EC, D], BF16, kind="ExternalInput")
    cos2_d = nc.dram_tensor("cos2", [128, T], F32, kind="ExternalInput")
    sin2p_d = nc.dram_tensor("sin2p", [128, T], F32, kind="ExternalInput")
    tri_d = nc.dram_tensor("tri", [128, 128], BF16, kind="ExternalInput")
    ztri_d = nc.dram_tensor("ztri", [128, 256], BF16, kind="ExternalInput")
    ones_d = nc.dram_tensor("ones", [128, 1], BF16, kind="ExternalInput")
    ones4_d = nc.dram_tensor("ones4", [33, 64], mybir.dt.float32r,
                             kind="ExternalInput")
    outT_d = nc.dram_tensor("outT", [D, T], F32, kind="ExternalOutput")

    with tile.TileContext(nc) as tc:
        with tc.tile_pool(name="perm", bufs=1) as perm, \
             tc.tile_pool(name="psum", bufs=1, space="PSUM") as psp, \
             tc.tile_pool(name="rw", bufs=3) as rw, \
             tc.tile_pool(name="ew", bufs=3) as ew:
            # ---- persistent tiles
            x_sb = perm.tile([128, KT, T], BF16)
            wq_sb = perm.tile([128, KT, EC], BF16)
            wk_sb = perm.tile([128, KT, EC], BF16)
            wv_sb = perm.tile([128, KT, EC], BF16)
            wo_sb = perm.tile([128, 2, D], BF16)
            cos_sb = perm.tile([128, T], F32)
            sin_sb = perm.tile([128, T], F32)
            qT = perm.tile([128, PT, T], BF16)
            kT = perm.tile([128, PT, T], BF16)
            v_sb = perm.tile([128, NTT, HPC, 65], BF16)
            OT_all = perm.tile([128, PT, T], BF16)
            tri_sb = perm.tile([128, 128], BF16)
            ztri_sb = perm.tile([128, 256], BF16)
            ones4_sb = perm.tile([33, 64], mybir.dt.float32r)

            # ---- loads, in consumption order, alternating the two HWDGE rings
            qs = [nc.sync, nc.scalar]
            qi = 0

            def ld(out, in_):
                nonlocal qi
                qs[qi % 2].dma_start(out=out, in_=in_)
                qi += 1

            for k in range(KT):
                ld(wq_sb[:, k, :], wqT_d.ap()[128 * k:128 * k + 128, :])
            for k in range(KT):
                ld(x_sb[:, k, 0:512], xT_d.ap()[128 * k:128 * k + 128, 0:512])
            for k in range(KT):
                ld(wk_sb[:, k, :], wkT_d.ap()[128 * k:128 * k + 128, :])
            ld(cos_sb[:, 0:512], cos2_d.ap()[:, 0:512])
            ld(sin_sb[:, 0:512], sin2p_d.ap()[:, 0:512])
            for k in range(KT):
                ld(x_sb[:, k, 512:1024], xT_d.ap()[128 * k:128 * k + 128, 512:1024])
            for k in range(KT):
                ld(wv_sb[:, k, :], wvT_d.ap()[128 * k:128 * k + 128, :])
            ld(tri_sb, tri_d.ap())
            ld(ztri_sb, ztri_d.ap())
            ld(ones4_sb, ones4_d.ap())
            ones_bcast = bass.AP(tensor=ones_d, offset=0,
                                 ap=[[1, 128], [0, NTT * HPC]])
            ld(v_sb[:, :, :, 64:65].rearrange("p a b c -> p (a b c)"), ones_bcast)
            for k in range(KT):
                ld(x_sb[:, k, 1024:1536], xT_d.ap()[128 * k:128 * k + 128, 1024:1536])
            ld(cos_sb[:, 512:2048], cos2_d.ap()[:, 512:2048])
            ld(sin_sb[:, 512:2048], sin2p_d.ap()[:, 512:2048])
            for k in range(KT):
                ld(x_sb[:, k, 1536:2048], xT_d.ap()[128 * k:128 * k + 128, 1536:2048])
            for ct in range(2):
                ld(wo_sb[:, ct, :], woT_d.ap()[128 * ct:128 * ct + 128, :])

            # ================= phase 1: projections + RoPE + vT transpose ====
            for nb in range(NB):
                cols = slice(512 * nb, 512 * nb + 512)
                for w_sb, dst, wnm in ((wq_sb, qT, "q"), (wk_sb, kT, "k")):
                    for pt in range(PT):
                        ps = psp.tile([128, 512], F32, tag="mm", bufs=2,
                                      name=f"ps{wnm}{pt}{nb}")
                        for k in range(KT):
                            nc.tensor.matmul(
                                ps, w_sb[:, k, 128 * pt:128 * pt + 128],
                                x_sb[:, k, cols],
                                start=(k == 0), stop=(k == KT - 1))
                        qc = rw.tile([128, 512], F32, tag="qc")
                        qp = rw.tile([128, 512], F32, tag="qp")
                        shf = rw.tile([128, 512], F32, tag="shf")
                        nc.vector.tensor_mul(out=qc, in0=ps, in1=cos_sb[:, cols])
                        nc.vector.tensor_mul(out=qp, in0=ps, in1=sin_sb[:, cols])
                        # RoPE rotate-half: partition shuffle via single-input
                        # copies (out base may differ from in base), then a
                        # same-base add (two-SBUF-input ops must share bases)
                        for b2 in range(2):
                            base = 64 * b2
                            nc.vector.tensor_copy(
                                out=shf[base:base + 32, :],
                                in_=qp[base + 32:base + 64, :])
                            nc.vector.tensor_copy(
                                out=shf[base + 32:base + 64, :],
                                in_=qp[base:base + 32, :])
                        with nc.allow_low_precision(reason="bf16 q/k for PE"):
                            nc.vector.tensor_add(
                                out=dst[:, pt, cols], in0=qc, in1=shf)
                for c4 in range(4):
                    tt = 4 * nb + c4
                    ps = psp.tile([128, 256], F32, tag="mm", bufs=2,
                                  name=f"psv{tt}")
                    for k in range(KT):
                        nc.tensor.matmul(
                            ps, x_sb[:, k, 128 * tt:128 * tt + 128],
                            wv_sb[:, k, :],
                            start=(k == 0), stop=(k == KT - 1))
                    with nc.allow_low_precision(reason="bf16 v for PE"):
                        nc.vector.tensor_copy(
                            out=v_sb[:, tt, :, 0:64],
                            in_=ps.rearrange("p (h d) -> p h d", h=HPC))

            # ================= phase 2: attention + out_proj =================
            # normalize + out_proj of block N-1 are emitted after the
            # attention of block N, so the PE never stalls on the softmax
            # denominator chain (vector reciprocal) mid-stream.
            pending = {}

            def emit_normalize_outproj(pblk):
                p = pending.pop(pblk)
                pcols = slice(512 * pblk, 512 * pblk + 512)
                rc = p["rc"]
                for pair in range(PT):
                    for hd in range(2):
                        r = 2 * pair + hd
                        rcp = psp.tile([64, 512], F32, tag="ot", bufs=2,
                                       name=f"rcp{pblk}{pair}{hd}")
                        nc.tensor.matmul(rcp, ones4_sb[32 * hd:32 * hd + 1, :],
                                         rc[pair][32 * hd:32 * hd + 1, :],
                                         start=True, stop=True)
                        with nc.allow_low_precision(reason="bf16 attn out"):
                            nc.vector.tensor_mul(
                                out=OT_all[64 * hd:64 * hd + 64, pair, pcols],
                                in0=p["ots"][pair][hd][0:64, :], in1=rcp)
                for m in range(KT):
                    fp = psp.tile([128, 512], F32, tag="mm", bufs=2,
                                  name=f"fp{pblk}{m}")
                    for ct in range(2):
                        nc.tensor.matmul(
                            fp, wo_sb[:, ct, 128 * m:128 * m + 128],
                            OT_all[:, ct, pcols],
                            start=(ct == 0), stop=(ct == 1))
                    fs = ew.tile([128, 512], F32, tag="fs",
                                 name=f"fs{pblk}{m}")
                    nc.vector.tensor_copy(out=fs, in_=fp)
                    nc.scalar.dma_start(
                        out=outT_d.ap()[128 * m:128 * m + 128, pcols], in_=fs)

            for blk in range(NB):
                cols = slice(512 * blk, 512 * blk + 512)
                dall = [ew.tile([33, 512], F32, tag=f"dall{p}", bufs=2,
                                name=f"dall{blk}{p}") for p in range(PT)]
                for p in range(PT):
                    nc.gpsimd.memset(dall[p], 1.0)
                blk_ots = []
                for pair in range(PT):
                    ot = [psp.tile([65, 512], F32, tag="ot", bufs=2,
                                   name=f"ot{blk}{pair}{hd}")
                          for hd in range(2)]
                    ntk = 2 * (blk + 1)
                    last = (ntk - 1, 1)
                    for tkp in range(ntk):
                        t0 = 2 * tkp
                        lo = [max(0, 128 * (t0 + h - 4 * blk)) for h in (0, 1)]
                        lop = lo[0]
                        st = [psp.tile([128, 2, 512], F32, tag="st2", bufs=2,
                                       name=f"st{blk}{pair}{tkp}{hd}")
                              for hd in range(2)]
                        for hd in range(2):
                            hrow = slice(64 * hd, 64 * hd + 64)
                            for h in (0, 1):
                                tt = t0 + h
                                nc.tensor.matmul(
                                    st[hd][:, h, lop:512],
                                    kT[hrow, pair, 128 * tt:128 * tt + 128],
                                    qT[hrow, pair, 512 * blk + lop:512 * blk + 512],
                                    start=True, stop=True)
                        ex = [ew.tile([128, 2, 512], BF16, tag="ex", bufs=4,
                                      name=f"ex{blk}{pair}{tkp}{hd2}")
                              for hd2 in range(2)]
                        for hd in range(2):
                            with nc.allow_low_precision(reason="bf16 probs"):
                                nc.scalar.activation(
                                    out=ex[hd][:, :, lop:512],
                                    in_=st[hd][:, :, lop:512],
                                    func=mybir.ActivationFunctionType.Exp,
                                    scale=0.125)
                            # causal masks on diagonal tiles
                            for h in (0, 1):
                                j = t0 + h - 4 * blk
                                if j < 0:
                                    continue
                                lo_h = lo[h]
                                with nc.allow_low_precision(reason="bf16 mask"):
                                    if lo_h == lop:
                                        nc.vector.tensor_mul(
                                            out=ex[hd][:, h, lo_h:lo_h + 128],
                                            in0=ex[hd][:, h, lo_h:lo_h + 128],
                                            in1=tri_sb)
                                    else:
                                        w = lo_h + 128 - lop
                                        nc.vector.tensor_mul(
                                            out=ex[hd][:, h, lop:lo_h + 128],
                                            in0=ex[hd][:, h, lop:lo_h + 128],
                                            in1=ztri_sb[:, 0:w])
                            for h in (0, 1):
                                tt = t0 + h
                                lo_h = lo[h]
                                nc.tensor.matmul(
                                    ot[hd][:, lo_h:512],
                                    v_sb[:, tt, 2 * pair + hd, :],
                                    ex[hd][:, h, lo_h:512],
                                    start=(tkp == 0 and h == 0),
                                    stop=(tkp, h) == last)
                    # copy raw OT out of PSUM (frees the banks) and gather the
                    # ones-row denominators into dall rows
                    ots = [ew.tile([65, 512], F32, tag="ots", bufs=8,
                                   name=f"ots{blk}{pair}{hd}")
                           for hd in range(2)]
                    for hd in range(2):
                        nc.vector.tensor_copy(out=ots[hd], in_=ot[hd])
                        nc.vector.tensor_copy(
                            out=dall[pair][32 * hd:32 * hd + 1, :],
                            in_=ots[hd][64:65, :])
                    blk_ots.append(ots)
                # one batched reciprocal for this block's 4 denominator rows
                rc = [ew.tile([33, 512], mybir.dt.float32r, tag=f"rc{p}",
                              bufs=2, name=f"rc{blk}{p}") for p in range(PT)]
                # one call per pair covers both 32-aligned rows (cost is per-
                # partition elems; junk partitions between rows are unread)
                with nc.allow_low_precision(reason="f32r recip feeds bcast matmul"):
                    nc.vector.reciprocal(out=rc[0], in_=dall[0])
                    nc.vector.reciprocal(out=rc[1], in_=dall[1])
                pending[blk] = {"ots": blk_ots, "rc": rc}
                if blk > 0:
                    emit_normalize_outproj(blk - 1)
            emit_normalize_outproj(NB - 1)

    nc.compile()
    return nc


def _consts():
    i = np.arange(32)
    theta = 1.0 / (10000.0 ** (2.0 * i / 64))
    ang = np.outer(np.arange(T, dtype=np.float64), theta)
    p = np.arange(128)
    cos2 = np.cos(ang[:, p % 32]).T.astype(np.float32)
    sgn = np.where((p % 64) < 32, -1.0, 1.0)
    sin2s = (np.sin(ang[:, p % 32]) * sgn).T.astype(np.float32)
    cos2 = np.ascontiguousarray(cos2)
    # pre-shuffled sin so the kernel can multiply BEFORE the partition shuffle:
    # shuf(q * sin2p)[p] = q[p^32] * sin2s[p]
    sin2p = np.ascontiguousarray(sin2s[p ^ 32])
    r, c = np.meshgrid(np.arange(128), np.arange(128), indexing="ij")
    BF = mybir.dt.np(BF16)
    tri = (r <= c).astype(BF)
    ztri = np.ascontiguousarray(
        np.concatenate([np.zeros((128, 128)), (r <= c)], axis=1)).astype(BF)
    ones = np.ones((128, 1), BF)
    ones4 = np.ones((33, 64), np.float32)
    return cos2, sin2p, tri, ztri, ones, ones4


def kernel(x, Wq, Wk, Wv, Wo, _trace=False):
    BF = mybir.dt.np(BF16)
    x = np.asarray(x, dtype=np.float32)
    Wq = np.asarray(Wq, dtype=np.float32)
    Wk = np.asarray(Wk, dtype=np.float32)
    Wv = np.asarray(Wv, dtype=np.float32)
    Wo = np.asarray(Wo, dtype=np.float32)

    if "nc" not in _NC_CACHE:
        _NC_CACHE["nc"] = _build_nc()
    nc = _NC_CACHE["nc"]

    cos2, sin2p, tri, ztri, ones, ones4 = _consts()
    xTs = [np.ascontiguousarray(x[b].T).astype(BF) for b in range(B)]
    WqT, WkT, WvT, WoT = Wq.T, Wk.T, Wv.T, Wo.T

    in_maps = []
    for c in range(8):
        b, g = c // HG, c % HG
        cs = slice(EC * g, EC * g + EC)
        in_maps.append({
            "xT": xTs[b],
            "wqT": np.ascontiguousarray(WqT[:, cs]).astype(BF),
            "wkT": np.ascontiguousarray(WkT[:, cs]).astype(BF),
            "wvT": np.ascontiguousarray(WvT[:, cs]).astype(BF),
            "woT": np.ascontiguousarray(WoT[cs, :]).astype(BF),
            "cos2": cos2, "sin2p": sin2p,
            "tri": tri, "ztri": ztri, "ones": ones, "ones4": ones4,
        })

    kw = {}
    if _trace:
        kw = dict(trace=True, trace_cores=list(range(8)))
    res = run_bass_kernel_spmd(nc, in_maps, core_ids=list(range(8)), **kw)

    out = np.zeros((B, T, D), np.float32)
    for c in range(8):
        out[c // HG] += res.results[c]["outT"].T
    if _trace:
        return out, res
    return out


# revision 20
# speedup vs baseline: 1.4009x; 1.4009x over previous
"""Trainium2 Bass kernel for CachedRoPEAttention.

Sharding: 8 cores = batch(2) x head-groups(4). Each core computes 4 heads of
one batch element end-to-end (q/k/v proj in [e,t] layout, RoPE, causal
flash-style attention with ones-row softmax denominators, out_proj partial),
host sums the 4 tensor-parallel partials per batch.

All matmuls run in bf16 (inputs host-cast; PSUM accumulation stays f32).
Key perf structure vs the f32r baseline:
  - bf16 halves HBM traffic and keeps the PE at 1 cycle/row for every n.
  - v is produced as vT via weight-stationary matmuls (n=512, LDWEIGHTS
    hidden), then flipped to [t, e] layout by XBAR DMA transposes -- no PE
    or DVE cost.
  - The RoPE partition shuffle rides the final DVE add (cross-partition
    in1 offsets) instead of serial SBUF-SBUF DMAs on the sync queue.
  - Softmax denominators: DVE reciprocal_approx_fast on the [1,512] ones
    rows + DMA partition-broadcast, replacing the 3.3us full-precision
    reciprocals and the PE ones-outer-product broadcast.
  - x/w loads are issued in consumption order (wq, x-slab0, wk, ...) so the
    first projection matmul starts as early as possible.
"""
import sys
sys.path.insert(0, "/opt/trn_rl_repo")

import numpy as np

import concourse.bass as bass
import concourse.bacc as bacc
import concourse.mybir as mybir
import concourse.tile as tile
from concourse.bass_utils import run_bass_kernel_spmd

F32 = mybir.dt.float32
BF16 = mybir.dt.bfloat16

D, H, DH, T, B = 1024, 16, 64, 2048, 2
HG, HPC, EC = 4, 4, 256      # head groups, heads/core, e-width/core
KT = D // 128                # 8 contraction tiles over d_model
PT = EC // 128               # 2 e-partition-tiles (head pairs) per core
NB = T // 512                # 4 t-blocks
NTT = T // 128               # 16 t-tiles

_NC_CACHE = {}


def _build_nc():
    nc = bacc.Bacc(None, target_bir_lowering=False)

    xT_d = nc.dram_tensor("xT", [D, T], BF16, kind="ExternalInput")
    wqT_d = nc.dram_tensor("wqT", [D, EC], BF16, kind="ExternalInput")
    wkT_d = nc.dram_tensor("wkT", [D, EC], BF16, kind="ExternalInput")
    wvT_d = nc.dram_tensor("wvT", [D, EC], BF16, kind="ExternalInput")
    woT_d = nc.dram_tensor("woT", [EC, D], BF16, kind="ExternalInput")
    cos2_d = nc.dram_tensor("cos2", [128, T], F32, kind="ExternalInput")
    sin2p_d = nc.dram_tensor("sin2p", [128, T], F32, kind="ExternalInput")
    tri_d = nc.dram_tensor("tri", [128, 128], BF16, kind="ExternalInput")
    ztri_d = nc.dram_tensor("ztri", [128, 256], BF16, kind="ExternalInput")
    ones4_d = nc.dram_tensor("ones4", [33, 64], mybir.dt.float32r,
                             kind="ExternalInput")
    outT_d = nc.dram_tensor("outT", [D, T], F32, kind="ExternalOutput")

    with tile.TileContext(nc) as tc:
        with tc.tile_pool(name="perm", bufs=1) as perm, \
             tc.tile_pool(name="psum", bufs=1, space="PSUM") as psp, \
             tc.tile_pool(name="rw", bufs=3) as rw, \
             tc.tile_pool(name="ew", bufs=3) as ew:
            # ---- persistent tiles
            x_sb = perm.tile([128, KT, T], BF16)
            wq_sb = perm.tile([128, KT, EC], BF16)
            wk_sb = perm.tile([128, KT, EC], BF16)
            wv_sb = perm.tile([128, KT, EC], BF16)
            wo_sb = perm.tile([128, 2, D], BF16)
            cos_sb = perm.tile([128, T], F32)
            sin_sb = perm.tile([128, T], F32)
            qT = perm.tile([128, PT, T], BF16)
            kT = perm.tile([128, PT, T], BF16)
            v_sb = perm.tile([128, NTT, HPC, 65], BF16)
            OT_all = perm.tile([128, PT, T], BF16)
            tri_sb = perm.tile([128, 128], BF16)
            ztri_sb = perm.tile([128, 256], BF16)
            ones4_sb = perm.tile([33, 64], mybir.dt.float32r)

            # ---- loads, in consumption order, alternating the two HWDGE rings
            qs = [nc.sync, nc.scalar]
            qi = 0

            def ld(out, in_):
                nonlocal qi
                qs[qi % 2].dma_start(out=out, in_=in_)
                qi += 1

            # x loaded as full [128, 2048] rows: descriptor-gen on the
            # issuing sequencers is ~20ns/row-descriptor, so fewer, larger
            # lines keep the rings free for compute-phase instructions
            for k in range(KT):
                ld(wq_sb[:, k, :], wqT_d.ap()[128 * k:128 * k + 128, :])
                ld(wk_sb[:, k, :], wkT_d.ap()[128 * k:128 * k + 128, :])
            for k in range(KT):
                ld(x_sb[:, k, :], xT_d.ap()[128 * k:128 * k + 128, :])
            ld(cos_sb, cos2_d.ap())
            ld(sin_sb, sin2p_d.ap())
            for k in range(KT):
                ld(wv_sb[:, k, :], wvT_d.ap()[128 * k:128 * k + 128, :])
            ld(tri_sb, tri_d.ap())
            ld(ztri_sb, ztri_d.ap())
            ld(ones4_sb, ones4_d.ap())
            # ones column of v (softmax denominator trick): engine memset, a
            # broadcast DMA here costs ~70us of serial descriptor processing
            nc.gpsimd.memset(
                v_sb[:, :, :, 64:65].rearrange("p a b c -> p (a b c)"), 1.0)
            for ct in range(2):
                ld(wo_sb[:, ct, :], woT_d.ap()[128 * ct:128 * ct + 128, :])

            # ================= phase 1: projections + RoPE + vT transpose ====
            for nb in range(NB):
                cols = slice(512 * nb, 512 * nb + 512)
                for w_sb, dst, wnm in ((wq_sb, qT, "q"), (wk_sb, kT, "k")):
                    for pt in range(PT):
                        ps = psp.tile([128, 512], F32, tag="mm", bufs=2,
                                      name=f"ps{wnm}{pt}{nb}")
                        for k in range(KT):
                            nc.tensor.matmul(
                                ps, w_sb[:, k, 128 * pt:128 * pt + 128],
                                x_sb[:, k, cols],
                                start=(k == 0), stop=(k == KT - 1))
                        qc = rw.tile([128, 512], F32, tag="qc")
                        qp = rw.tile([128, 512], F32, tag="qp")
                        shf = rw.tile([128, 512], F32, tag="shf")
                        nc.vector.tensor_mul(out=qc, in0=ps, in1=cos_sb[:, cols])
                        nc.vector.tensor_mul(out=qp, in0=ps, in1=sin_sb[:, cols])
                        # RoPE rotate-half: partition shuffle via single-input
                        # copies (out base may differ from in base), then a
                        # same-base add (two-SBUF-input ops must share bases)
                        for b2 in range(2):
                            base = 64 * b2
                            nc.vector.tensor_copy(
                                out=shf[base:base + 32, :],
                                in_=qp[base + 32:base + 64, :])
                            nc.vector.tensor_copy(
                                out=shf[base + 32:base + 64, :],
                                in_=qp[base:base + 32, :])
                        with nc.allow_low_precision(reason="bf16 q/k for PE"):
                            nc.vector.tensor_add(
                                out=dst[:, pt, cols], in0=qc, in1=shf)
                for c4 in range(4):
                    tt = 4 * nb + c4
                    ps = psp.tile([128, 256], F32, tag="st2", bufs=2,
                                  name=f"psv{tt}")
                    for k in range(KT):
                        nc.tensor.matmul(
                            ps, x_sb[:, k, 128 * tt:128 * tt + 128],
                            wv_sb[:, k, :],
                            start=(k == 0), stop=(k == KT - 1))
                    with nc.allow_low_precision(reason="bf16 v for PE"):
                        nc.vector.tensor_copy(
                            out=v_sb[:, tt, :, 0:64],
                            in_=ps.rearrange("p (h d) -> p h d", h=HPC))

            # ================= phase 2: attention + out_proj =================
            # normalize + out_proj of block N-1 are emitted after the
            # attention of block N, so the PE never stalls on the softmax
            # denominator chain (vector reciprocal) mid-stream.
            pending = {}

            def emit_normalize_outproj(pblk):
                p = pending.pop(pblk)
                pcols = slice(512 * pblk, 512 * pblk + 512)
                rc = p["rc"]
                for pair in range(PT):
                    for hd in range(2):
                        r = 2 * pair + hd
                        rcp = psp.tile([64, 512], F32, tag="ot", bufs=2,
                                       name=f"rcp{pblk}{pair}{hd}")
                        nc.tensor.matmul(rcp, ones4_sb[32 * hd:32 * hd + 1, :],
                                         rc[pair][32 * hd:32 * hd + 1, :],
                                         start=True, stop=True)
                        with nc.allow_low_precision(reason="bf16 attn out"):
                            nc.vector.tensor_mul(
                                out=OT_all[64 * hd:64 * hd + 64, pair, pcols],
                                in0=p["ots"][pair][hd][0:64, :], in1=rcp)
                for m in range(KT):
                    fp = psp.tile([128, 512], F32, tag="mm", bufs=2,
                                  name=f"fp{pblk}{m}")
                    for ct in range(2):
                        nc.tensor.matmul(
                            fp, wo_sb[:, ct, 128 * m:128 * m + 128],
                            OT_all[:, ct, pcols],
                            start=(ct == 0), stop=(ct == 1))
                    fs = ew.tile([128, 512], F32, tag="fs",
                                 name=f"fs{pblk}{m}")
                    nc.vector.tensor_copy(out=fs, in_=fp)
                    nc.sync.dma_start(
                        out=outT_d.ap()[128 * m:128 * m + 128, pcols], in_=fs)

            for blk in range(NB):
                cols = slice(512 * blk, 512 * blk + 512)
                dall = [ew.tile([33, 512], F32, tag=f"dall{p}", bufs=2,
                                name=f"dall{blk}{p}") for p in range(PT)]
                for p in range(PT):
                    nc.gpsimd.memset(dall[p], 1.0)
                blk_ots = []
                for pair in range(PT):
                    ot = [psp.tile([65, 512], F32, tag="ot", bufs=2,
                                   name=f"ot{blk}{pair}{hd}")
                          for hd in range(2)]
                    ntk = 2 * (blk + 1)
                    last = (ntk - 1, 1)
                    for tkp in range(ntk):
                        t0 = 2 * tkp
                        lo = [max(0, 128 * (t0 + h - 4 * blk)) for h in (0, 1)]
                        lop = lo[0]
                        st = [psp.tile([128, 2, 512], F32, tag="st2", bufs=2,
                                       name=f"st{blk}{pair}{tkp}{hd}")
                              for hd in range(2)]
                        for hd in range(2):
                            hrow = slice(64 * hd, 64 * hd + 64)
                            for h in (0, 1):
                                tt = t0 + h
                                nc.tensor.matmul(
                                    st[hd][:, h, lop:512],
                                    kT[hrow, pair, 128 * tt:128 * tt + 128],
                                    qT[hrow, pair, 512 * blk + lop:512 * blk + 512],
                                    start=True, stop=True)
                        ex = [ew.tile([128, 2, 512], BF16, tag="ex", bufs=4,
                                      name=f"ex{blk}{pair}{tkp}{hd2}")
                              for hd2 in range(2)]
                        for hd in range(2):
                            with nc.allow_low_precision(reason="bf16 probs"):
                                nc.scalar.activation(
                                    out=ex[hd][:, :, lop:512],
                                    in_=st[hd][:, :, lop:512],
                                    func=mybir.ActivationFunctionType.Exp,
                                    scale=0.125)
                            # causal masks on diagonal tiles
                            for h in (0, 1):
                                j = t0 + h - 4 * blk
                                if j < 0:
                                    continue
                                lo_h = lo[h]
                                with nc.allow_low_precision(reason="bf16 mask"):
                                    if lo_h == lop:
                                        nc.vector.tensor_mul(
                                            out=ex[hd][:, h, lo_h:lo_h + 128],
                                            in0=ex[hd][:, h, lo_h:lo_h + 128],
                                            in1=tri_sb)
                                    else:
                                        w = lo_h + 128 - lop
                                        nc.vector.tensor_mul(
                                            out=ex[hd][:, h, lop:lo_h + 128],
                                            in0=ex[hd][:, h, lop:lo_h + 128],
                                            in1=ztri_sb[:, 0:w])
                            for h in (0, 1):
                                tt = t0 + h
                                lo_h = lo[h]
                                nc.tensor.matmul(
                                    ot[hd][:, lo_h:512],
                                    v_sb[:, tt, 2 * pair + hd, :],
                                    ex[hd][:, h, lo_h:512],
                                    start=(tkp == 0 and h == 0),
                                    stop=(tkp, h) == last)
                    # copy raw OT out of PSUM (frees the banks) and gather the
                    # ones-row denominators into dall rows
                    ots = [ew.tile([65, 512], F32, tag="ots", bufs=8,
                                   name=f"ots{blk}{pair}{hd}")
                           for hd in range(2)]
                    for hd in range(2):
                        nc.vector.tensor_copy(out=ots[hd], in_=ot[hd])
                        nc.vector.tensor_copy(
                            out=dall[pair][32 * hd:32 * hd + 1, :],
                            in_=ots[hd][64:65, :])
                    blk_ots.append(ots)
                # one batched reciprocal for this block's 4 denominator rows
                rc = [ew.tile([33, 512], mybir.dt.float32r, tag=f"rc{p}",
                              bufs=2, name=f"rc{blk}{p}") for p in range(PT)]
                # one call per pair covers both 32-aligned rows (cost is per-
                # partition elems; junk partitions between rows are unread)
                with nc.allow_low_precision(reason="f32r recip feeds bcast matmul"):
                    nc.vector.reciprocal(out=rc[0], in_=dall[0])
                    nc.vector.reciprocal(out=rc[1], in_=dall[1])
                pending[blk] = {"ots": blk_ots, "rc": rc}
                if blk > 0:
                    emit_normalize_outproj(blk - 1)
            emit_normalize_outproj(NB - 1)

    nc.compile()
    return nc


def _consts():
    i = np.arange(32)
    theta = 1.0 / (10000.0 ** (2.0 * i / 64))
    ang = np.outer(np.arange(T, dtype=np.float64), theta)
    p = np.arange(128)
    cos2 = np.cos(ang[:, p % 32]).T.astype(np.float32)
    sgn = np.where((p % 64) < 32, -1.0, 1.0)
    sin2s = (np.sin(ang[:, p % 32]) * sgn).T.astype(np.float32)
    cos2 = np.ascontiguousarray(cos2)
    # pre-shuffled sin so the kernel can multiply BEFORE the partition shuffle:
    # shuf(q * sin2p)[p] = q[p^32] * sin2s[p]
    sin2p = np.ascontiguousarray(sin2s[p ^ 32])
    r, c = np.meshgrid(np.arange(128), np.arange(128), indexing="ij")
    BF = mybir.dt.np(BF16)
    tri = (r <= c).astype(BF)
    ztri = np.ascontiguousarray(
        np.concatenate([np.zeros((128, 128)), (r <= c)], axis=1)).astype(BF)
    ones4 = np.ones((33, 64), np.float32)
    return cos2, sin2p, tri, ztri, ones4


def kernel(x, Wq, Wk, Wv, Wo, _trace=False):
    BF = mybir.dt.np(BF16)
    x = np.asarray(x, dtype=np.float32)
    Wq = np.asarray(Wq, dtype=np.float32)
    Wk = np.asarray(Wk, dtype=np.float32)
    Wv = np.asarray(Wv, dtype=np.float32)
    Wo = np.asarray(Wo, dtype=np.float32)

    if "nc" not in _NC_CACHE:
        _NC_CACHE["nc"] = _build_nc()
    nc = _NC_CACHE["nc"]

    cos2, sin2p, tri, ztri, ones4 = _consts()
    xTs = [np.ascontiguousarray(x[b].T).astype(BF) for b in range(B)]
    WqT, WkT, WvT, WoT = Wq.T, Wk.T, Wv.T, Wo.T

    in_maps = []
    for c in range(8):
        b, g = c // HG, c % HG
        cs = slice(EC * g, EC * g + EC)
        in_maps.append({
            "xT": xTs[b],
            "wqT": np.ascontiguousarray(WqT[:, cs]).astype(BF),
            "wkT": np.ascontiguousarray(WkT[:, cs]).astype(BF),
            "wvT": np.ascontiguousarray(WvT[:, cs]).astype(BF),
            "woT": np.ascontiguousarray(WoT[cs, :]).astype(BF),
            "cos2": cos2, "sin2p": sin2p,
            "tri": tri, "ztri": ztri, "ones4": ones4,
        })

    kw = {}
    if _trace:
        kw = dict(trace=True, trace_cores=list(range(8)))
    res = run_bass_kernel_spmd(nc, in_maps, core_ids=list(range(8)), **kw)

    out = np.zeros((B, T, D), np.float32)
    for c in range(8):
        out[c // HG] += res.results[c]["outT"].T
    if _trace:
        return out, res
    return out
